# revision 5
# baseline (speedup 1.0000x reference)
"""Trainium2 Bass kernel for the Dempster-Shafer sequential-combination layer.

Math (per batch element; inputs m[p, k], p=0..63 prototypes, k=0..10 with
slot 10 = omega):
    The reference left-fold is  M' = M*(m + w) + M_w*m  applied uniformly to
    all 11 slots (so the omega slot picks up a 3x factor per step), followed
    by a per-step normalization.  Normalization is a uniform positive scale
    and the step map is linear in M, so every intermediate normalization
    cancels in the final one.  Rescaling the state by the running omega
    product (y = M / M_w) turns the fold into
        y' = (Q + 1/3) * y + Q,      Q[p, k] = m[p, k] / (3 * w[p])
    with y_0 = m[0]/w[0] and y_omega == 1 identically, so the final output is
        out_k = y_k(63) / (1 + sum_{k<10} y_k(63)),   out_omega = 1 / (...)
    and neither the omega product nor the omega chains need computing.  This
    maps exactly onto the DVE tensor_tensor_scan instruction
    (state = d0*state + d1) with per-(batch,k) chains laid p-contiguous along
    the free dimension; a zero in d0 at each chain start resets the
    recurrence across chain boundaries.

Engine split per chunk: DMA in (sync/HWDGE) -> ACT gathers 3w+delta (strided)
-> DVE reciprocal -> Q-build (transposing mul, split DVE/GpSimd by group) ->
ACT d0 = Q + 1/3 -> GpSimd zeroes chain starts -> DVE scan -> DVE epilogue
(reduce, 1/(S+1), final scale) -> DMA out.
"""

import numpy as np

B = 262144
P = 64
K = 11
KC = K - 1             # chains per group actually scanned (omega chain == 1)
N_CORES = 8
B_CORE = B // N_CORES  # 32768
NB = 8                 # batch groups per partition per chunk
CHUNK = 128 * NB
N_CHUNKS = B_CORE // CHUNK
DELTA = 1e-12          # guards w == 0 (present in uniform data)
THIRD = float(np.float32(1.0) / np.float32(3.0))
GSPLIT = 3             # groups 0..GSPLIT-1 of Q-build on DVE, rest on GpSimd

_CACHE = {}


def _build_program(reps=1, gsplit=GSPLIT, nb=NB):
    import concourse.bacc as bacc
    import concourse.mybir as mybir
    from concourse.tile import TileContext

    f32 = mybir.dt.float32
    Alu = mybir.AluOpType
    Act = mybir.ActivationFunctionType

    n_chunks = B_CORE // (128 * nb)
    nc = bacc.Bacc(
        "TRN2", target_bir_lowering=False, debug=False, num_devices=N_CORES
    )
    x = nc.declare_dram_parameter("x", [B_CORE, P * K], f32, isOutput=False)
    out = nc.declare_dram_parameter("out", [B_CORE, K], f32, isOutput=True)

    xv = x.rearrange("(c i g) d -> c i (g d)", i=128, g=nb)
    ov = out.rearrange("(c i g) d -> c i (g d)", i=128, g=nb)

    with TileContext(nc) as tc:
        with tc.tile_pool(name="p", bufs=2) as pool:
            for _rep in range(reps):
                for c in range(n_chunks):
                    m_ = pool.tile([128, nb * P * K], f32, name="m_")
                    u_ = pool.tile([128, nb * P], f32, name="u_")
                    u2_ = pool.tile([128, nb * P], f32, name="u2_")
                    q_ = pool.tile([128, nb * KC * P], f32, name="q_")
                    d0_ = pool.tile([128, nb * KC * P], f32, name="d0_")
                    y_ = pool.tile([128, nb * KC * P], f32, name="y_")
                    s_ = pool.tile([128, nb], f32, name="s_")
                    r_ = pool.tile([128, nb], f32, name="r_")
                    o_ = pool.tile([128, nb * K], f32, name="o_")

                    nc.sync.dma_start(out=m_[:], in_=xv[c])

                    m4 = m_.rearrange("p (g q k) -> p g q k", g=nb, q=P, k=K)
                    u3 = u_.rearrange("p (g q) -> p g q", g=nb)
                    # u_ = 3*w + delta (p>=1);  w + delta (p=0)
                    nc.scalar.activation(
                        out=u3[:, :, 1:], in_=m4[:, :, 1:, K - 1],
                        func=Act.Copy, bias=DELTA, scale=3.0,
                    )
                    nc.scalar.activation(
                        out=u3[:, :, 0:1], in_=m4[:, :, 0:1, K - 1],
                        func=Act.Copy, bias=DELTA, scale=1.0,
                    )
                    nc.vector.reciprocal(out=u2_[:], in_=u_[:])

                    # Q[g, k, p] = m[g, p, k] * u[g, p]  (chain layout, p inner,
                    # k = 0..9 only).  Split by group between DVE and GpSimd.
                    q4 = q_.rearrange("p (g k q) -> p g k q", g=nb, k=KC, q=P)
                    m4t = m4.transpose([0, 1, 3, 2])  # [128, nb, K, P]
                    u4b = (
                        u2_.rearrange("p (g q) -> p g q", g=nb)
                        .unsqueeze(2)
                        .broadcast_to([128, nb, KC, P])
                    )
                    gs = max(0, min(nb, gsplit))
                    if gs > 0:
                        nc.vector.tensor_tensor(
                            out=q4[:, :gs], in0=m4t[:, :gs, :KC],
                            in1=u4b[:, :gs], op=Alu.mult,
                        )
                    if gs < nb:
                        nc.gpsimd.tensor_tensor(
                            out=q4[:, gs:], in0=m4t[:, gs:, :KC],
                            in1=u4b[:, gs:], op=Alu.mult,
                        )

                    # d0 = Q + 1/3, with 0 at each chain start (p == 0)
                    nc.scalar.activation(
                        out=d0_[:], in_=q_[:], func=Act.Copy,
                        bias=THIRD, scale=1.0,
                    )
                    d04 = d0_.rearrange("p (g k q) -> p g k q", g=nb, k=KC, q=P)
                    nc.gpsimd.memset(d04[:, :, :, 0:1], 0.0)

                    # y[t] = d0[t] * y[t-1] + Q[t]
                    nc.vector.tensor_tensor_scan(
                        out=y_[:], data0=d0_[:], data1=q_[:],
                        initial=0.0, op0=Alu.mult, op1=Alu.add,
                    )

                    y4 = y_.rearrange("p (g k q) -> p g k q", g=nb, k=KC, q=P)
                    fin = y4[:, :, :, P - 1]  # [128, nb, KC]
                    nc.vector.tensor_reduce(
                        out=s_[:], in_=fin, axis=mybir.AxisListType.X,
                        op=Alu.add,
                    )
                    # r = 1 / (S + 1)   (the omega chain contributes exactly 1)
                    nc.vector.tensor_scalar(
                        out=s_[:], in0=s_[:], scalar1=1.0, scalar2=None,
                        op0=Alu.add,
                    )
                    nc.vector.reciprocal(out=r_[:], in_=s_[:])
                    rb = (
                        r_.rearrange("p g -> p g")
                        .unsqueeze(2)
                        .broadcast_to([128, nb, KC])
                    )
                    o3 = o_.rearrange("p (g k) -> p g k", g=nb)
                    nc.vector.tensor_tensor(
                        out=o3[:, :, :KC], in0=fin, in1=rb, op=Alu.mult
                    )
                    nc.vector.tensor_copy(out=o3[:, :, KC:], in_=r_[:].unsqueeze(2))

                    nc.sync.dma_start(out=ov[c], in_=o_[:])

    nc.compile()
    return nc


def kernel(inputs: np.ndarray) -> np.ndarray:
    from concourse.bass_utils import run_bass_kernel_spmd

    if "nc" not in _CACHE:
        _CACHE["nc"] = _build_program()
    nc = _CACHE["nc"]

    x = np.ascontiguousarray(np.asarray(inputs, dtype=np.float32)).reshape(
        B, P * K
    )
    shards = x.reshape(N_CORES, B_CORE, P * K)
    in_maps = [{"x": shards[i]} for i in range(N_CORES)]
    res = run_bass_kernel_spmd(nc, in_maps, core_ids=list(range(N_CORES)))
    outs = [res.results[i]["out"] for i in range(N_CORES)]
    return np.concatenate(outs, axis=0).reshape(B, K)
